# revision 3
# baseline (speedup 1.0000x reference)
"""Trainium2 Bass kernel for InformationPropagationLayer (GNN message passing).

Computes, per channel c:
    out[c] = 0.5 * h_in[c] + 0.5 * dinv[c] * (A[c] @ x[c] + x[c])
where dinv[c][n] = 1 / (1 + rowsum(A[c])[n])   (== D_tilde_inv @ (A + I) row-scaling)

Strategy:
  - C=16 channels sharded 2-per-core across 8 NeuronCores (no communication).
  - Host-side: A is transposed (so the contraction dim m lands on SBUF
    partitions with fully-contiguous DMA loads) and cast to bf16 (halves the
    dominant HBM traffic; final output error ~1e-5 because the output is
    dominated by the f32 h_in term).
  - x is augmented with a ones column, so a single matmul pass over A yields
    both (A @ x)^T and rowsum(A) (the ones column picks up sum_m A[n, m]).
  - Device: for each channel, accumulate psum[65, 512] = x_aug^T-chunks @ A^T
    over 16 K-chunks (4 psum banks = the 4 n-blocks of 512), then PE-transpose
    back to [n, 65] tiles and apply the fused epilogue on DVE.
"""

import numpy as np
import ml_dtypes
from contextlib import ExitStack

C, N, D = 16, 2048, 64
NCORES = 8
CPC = C // NCORES  # channels per core
P = 128            # SBUF partitions / matmul K
KT = N // P        # 16 contraction chunks
NBLK = 512         # psum bank free-dim (f32)
NBLKS = N // NBLK  # 4
DA = D + 1         # x columns + ones column
BF16 = ml_dtypes.bfloat16

_nc_cache = None


def _build():
    import concourse.mybir as mybir
    import concourse.tile as tile
    from concourse import bacc
    from concourse.masks import make_identity

    nc = bacc.Bacc(
        "TRN2",
        target_bir_lowering=False,
        debug=False,
        enable_asserts=True,
        num_devices=NCORES,
    )
    at = nc.dram_tensor("at", [CPC, N, N], mybir.dt.bfloat16, kind="ExternalInput").ap()
    xaug = nc.dram_tensor(
        "xaug", [CPC, N, DA], mybir.dt.bfloat16, kind="ExternalInput"
    ).ap()
    hh = nc.dram_tensor("hh", [CPC, N, D], mybir.dt.float32, kind="ExternalInput").ap()
    out = nc.dram_tensor("out", [CPC, N, D], mybir.dt.float32, kind="ExternalOutput").ap()

    f32 = mybir.dt.float32
    bf16 = mybir.dt.bfloat16
    mult = mybir.AluOpType.mult
    add = mybir.AluOpType.add

    with tile.TileContext(nc) as tc, ExitStack() as ctx:
        singles = ctx.enter_context(tc.tile_pool(name="singles", bufs=1))
        xpool = ctx.enter_context(tc.tile_pool(name="xpool", bufs=2))
        hpool = ctx.enter_context(tc.tile_pool(name="hpool", bufs=2))
        opool = ctx.enter_context(tc.tile_pool(name="opool", bufs=2))
        apool = ctx.enter_context(tc.tile_pool(name="apool", bufs=4))
        tpool = ctx.enter_context(tc.tile_pool(name="tpool", bufs=2))
        epool = ctx.enter_context(tc.tile_pool(name="epool", bufs=4))
        spool = ctx.enter_context(tc.tile_pool(name="spool", bufs=8))
        psacc = ctx.enter_context(tc.tile_pool(name="psacc", bufs=4, space="PSUM"))
        pstp = ctx.enter_context(tc.tile_pool(name="pstp", bufs=4, space="PSUM"))

        ident = singles.tile([P, P], f32)
        make_identity(nc, ident)

        for c in range(CPC):
            xs = xpool.tile([P, KT, DA], bf16, tag="xs")
            nc.sync.dma_start(out=xs, in_=xaug[c].rearrange("(k p) a -> p k a", p=P))
            hs = hpool.tile([P, KT, D], f32, tag="hs")
            nc.sync.dma_start(out=hs, in_=hh[c].rearrange("(k p) d -> p k d", p=P))
            os_ = opool.tile([P, KT, D], f32, tag="os")

            phs = [
                psacc.tile([DA, NBLK], f32, tag="ph", name=f"ph_{c}_{b}")
                for b in range(NBLKS)
            ]
            for ki in range(KT):
                a_sb = apool.tile([P, N], bf16, tag="a")
                nc.sync.dma_start(out=a_sb, in_=at[c, ki * P : (ki + 1) * P, :])
                for b in range(NBLKS):
                    nc.tensor.matmul(
                        phs[b][:, :],
                        xs[:, ki, :],
                        a_sb[:, b * NBLK : (b + 1) * NBLK],
                        start=(ki == 0),
                        stop=(ki == KT - 1),
                    )
            for b in range(NBLKS):
                hT = tpool.tile([DA, NBLK], f32, tag="hT")
                nc.vector.tensor_copy(hT[:, :], phs[b][:, :])
                for j in range(NBLK // P):
                    ti = b * (NBLK // P) + j
                    pt = pstp.tile([P, DA], f32, tag="pt")
                    nc.tensor.transpose(
                        pt[:, :], hT[:, j * P : (j + 1) * P], ident[:DA, :DA]
                    )
                    # den = 2 + 2*rowsum ; rcp = 1/den = 0.5 * dinv
                    den = spool.tile([P, 1], f32, tag="den")
                    nc.vector.tensor_scalar(den, pt[:, D:DA], 2.0, 2.0, op0=mult, op1=add)
                    rcp = spool.tile([P, 1], f32, tag="rcp")
                    nc.vector.reciprocal(rcp, den)
                    # hx = (A@x + x) * (0.5*dinv) ; out = hx + 0.5*h_in
                    hx = epool.tile([P, D], f32, tag="hx")
                    nc.vector.tensor_add(hx, pt[:, 0:D], xs[:, ti, 0:D])
                    nc.vector.tensor_scalar_mul(hx, hx, rcp)
                    nc.vector.tensor_add(os_[:, ti, :], hx, hs[:, ti, :])
            nc.sync.dma_start(out=out[c].rearrange("(k p) d -> p k d", p=P), in_=os_)

    nc.compile()
    return nc


def _get_nc():
    global _nc_cache
    if _nc_cache is None:
        _nc_cache = _build()
    return _nc_cache


def _make_in_maps(x, h_in, A_inter):
    x = np.asarray(x, dtype=np.float32)
    h_in = np.asarray(h_in, dtype=np.float32)
    A = np.asarray(A_inter, dtype=np.float32)
    ones = np.ones((CPC, N, 1), dtype=np.float32)
    in_maps = []
    for k in range(NCORES):
        cs = slice(CPC * k, CPC * (k + 1))
        at = A[cs].transpose(0, 2, 1).astype(BF16)
        xa = np.concatenate([x[cs], ones], axis=2).astype(BF16)
        hhalf = (0.5 * h_in[cs]).astype(np.float32)
        in_maps.append({"at": at, "xaug": xa, "hh": hhalf})
    return in_maps


def _run(in_maps, **kwargs):
    from concourse.bass_utils import run_bass_kernel_spmd

    nc = _get_nc()
    return run_bass_kernel_spmd(nc, in_maps, core_ids=list(range(NCORES)), **kwargs)


def kernel(x, h_in, A_inter):
    res = _run(_make_in_maps(x, h_in, A_inter))
    out = np.concatenate([res.results[k]["out"] for k in range(NCORES)], axis=0)
    return np.ascontiguousarray(out.astype(np.float32))


# revision 6
# speedup vs baseline: 1.0246x; 1.0246x over previous
"""Trainium2 Bass kernel for InformationPropagationLayer (GNN message passing).

Computes, per channel c:
    out[c] = 0.5 * h_in[c] + 0.5 * dinv[c] * (A[c] @ x[c] + x[c])
where dinv[c][n] = 1 / (1 + rowsum(A[c])[n])   (== D_tilde_inv @ (A + I) row-scaling)

Strategy:
  - C=16 channels sharded 2-per-core across 8 NeuronCores (no communication).
  - Host-side: A is transposed (so the contraction dim m lands on SBUF
    partitions with fully-contiguous DMA loads) and cast to bf16 (halves the
    dominant HBM traffic; final output error ~1e-5 because the output is
    dominated by the f32 h_in term).
  - x is augmented with a ones column, so a single matmul pass over A yields
    both (A @ x)^T and rowsum(A) (the ones column picks up sum_m A[n, m]).
  - Device: for each channel, accumulate psum[65, 512] = x_aug^T-chunks @ A^T
    over 16 K-chunks (4 psum banks = the 4 n-blocks of 512), then PE-transpose
    back to [n, 65] tiles and apply the fused epilogue on DVE.
"""

import numpy as np
import ml_dtypes
from contextlib import ExitStack

C, N, D = 16, 2048, 64
NCORES = 8
CPC = C // NCORES  # channels per core
P = 128            # SBUF partitions / matmul K
KT = N // P        # 16 contraction chunks
NBLK = 512         # psum bank free-dim (f32)
NBLKS = N // NBLK  # 4
DA = D + 1         # x columns + ones column
BF16 = ml_dtypes.bfloat16

_nc_cache = None


def _build():
    import concourse.mybir as mybir
    import concourse.tile as tile
    from concourse import bacc
    from concourse.masks import make_identity

    nc = bacc.Bacc(
        "TRN2",
        target_bir_lowering=False,
        debug=False,
        enable_asserts=True,
        num_devices=NCORES,
    )
    at = nc.dram_tensor("at", [CPC, N, N], mybir.dt.bfloat16, kind="ExternalInput").ap()
    xaug = nc.dram_tensor(
        "xaug", [CPC, N, DA], mybir.dt.bfloat16, kind="ExternalInput"
    ).ap()
    hh = nc.dram_tensor("hh", [CPC, N, D], mybir.dt.float32, kind="ExternalInput").ap()
    out = nc.dram_tensor("out", [CPC, N, D], mybir.dt.float32, kind="ExternalOutput").ap()

    f32 = mybir.dt.float32
    bf16 = mybir.dt.bfloat16
    mult = mybir.AluOpType.mult
    add = mybir.AluOpType.add

    with tile.TileContext(nc) as tc, ExitStack() as ctx:
        singles = ctx.enter_context(tc.tile_pool(name="singles", bufs=1))
        xpool = ctx.enter_context(tc.tile_pool(name="xpool", bufs=2))
        hpool = ctx.enter_context(tc.tile_pool(name="hpool", bufs=2))
        opool = ctx.enter_context(tc.tile_pool(name="opool", bufs=2))
        apool = ctx.enter_context(tc.tile_pool(name="apool", bufs=2))
        tpool = ctx.enter_context(tc.tile_pool(name="tpool", bufs=2))
        epool = ctx.enter_context(tc.tile_pool(name="epool", bufs=4))
        spool = ctx.enter_context(tc.tile_pool(name="spool", bufs=8))
        psacc = ctx.enter_context(tc.tile_pool(name="psacc", bufs=3, space="PSUM"))
        pstp = ctx.enter_context(tc.tile_pool(name="pstp", bufs=4, space="PSUM"))

        ident = singles.tile([P, P], f32)
        make_identity(nc, ident)

        # Round-robin big loads across engine HWDGE queues — a single queue
        # caps at ~260 GB/s, well under the core's HBM bandwidth.
        qs = [nc.sync, nc.scalar, nc.gpsimd]

        for c in range(CPC):
            xs = xpool.tile([P, KT, DA], bf16, tag="xs")
            nc.scalar.dma_start(out=xs, in_=xaug[c].rearrange("(k p) a -> p k a", p=P))
            hs = hpool.tile([P, KT, D], f32, tag="hs")
            nc.sync.dma_start(out=hs, in_=hh[c].rearrange("(k p) d -> p k d", p=P))
            os_ = opool.tile([P, KT, D], f32, tag="os")

            # Whole channel's A^T resident in SBUF (64KB/partition), loads
            # spread over all queues; frees PE to run 64 dense matmuls.
            a_all = apool.tile([P, KT, N], bf16, tag="a")
            for ki in range(KT):
                qs[ki % len(qs)].dma_start(
                    out=a_all[:, ki, :], in_=at[c, ki * P : (ki + 1) * P, :]
                )

            for b in range(NBLKS):
                ph = psacc.tile([DA, NBLK], f32, tag="ph")
                for ki in range(KT):
                    nc.tensor.matmul(
                        ph[:, :],
                        xs[:, ki, :],
                        a_all[:, ki, b * NBLK : (b + 1) * NBLK],
                        start=(ki == 0),
                        stop=(ki == KT - 1),
                    )
                hT = tpool.tile([DA, NBLK], f32, tag="hT")
                nc.vector.tensor_copy(hT[:, :], ph[:, :])
                for j in range(NBLK // P):
                    ti = b * (NBLK // P) + j
                    pt = pstp.tile([P, DA], f32, tag="pt")
                    nc.tensor.transpose(
                        pt[:, :], hT[:, j * P : (j + 1) * P], ident[:DA, :DA]
                    )
                    # den = 2 + 2*rowsum ; rcp = 1/den = 0.5 * dinv
                    den = spool.tile([P, 1], f32, tag="den")
                    nc.vector.tensor_scalar(den, pt[:, D:DA], 2.0, 2.0, op0=mult, op1=add)
                    rcp = spool.tile([P, 1], f32, tag="rcp")
                    nc.vector.reciprocal(rcp, den)
                    # hx = (A@x + x) * (0.5*dinv) ; out = hx + 0.5*h_in
                    hx = epool.tile([P, D], f32, tag="hx")
                    nc.vector.tensor_add(hx, pt[:, 0:D], xs[:, ti, 0:D])
                    nc.vector.tensor_scalar_mul(hx, hx, rcp)
                    nc.vector.tensor_add(os_[:, ti, :], hx, hs[:, ti, :])
            nc.gpsimd.dma_start(out=out[c].rearrange("(k p) d -> p k d", p=P), in_=os_)

    nc.compile()
    return nc


def _get_nc():
    global _nc_cache
    if _nc_cache is None:
        _nc_cache = _build()
    return _nc_cache


def _make_in_maps(x, h_in, A_inter):
    x = np.asarray(x, dtype=np.float32)
    h_in = np.asarray(h_in, dtype=np.float32)
    A = np.asarray(A_inter, dtype=np.float32)
    ones = np.ones((CPC, N, 1), dtype=np.float32)
    in_maps = []
    for k in range(NCORES):
        cs = slice(CPC * k, CPC * (k + 1))
        at = A[cs].transpose(0, 2, 1).astype(BF16)
        xa = np.concatenate([x[cs], ones], axis=2).astype(BF16)
        hhalf = (0.5 * h_in[cs]).astype(np.float32)
        in_maps.append({"at": at, "xaug": xa, "hh": hhalf})
    return in_maps


def _run(in_maps, **kwargs):
    from concourse.bass_utils import run_bass_kernel_spmd

    nc = _get_nc()
    return run_bass_kernel_spmd(nc, in_maps, core_ids=list(range(NCORES)), **kwargs)


def kernel(x, h_in, A_inter):
    res = _run(_make_in_maps(x, h_in, A_inter))
    out = np.concatenate([res.results[k]["out"] for k in range(NCORES)], axis=0)
    return np.ascontiguousarray(out.astype(np.float32))


# revision 13
# speedup vs baseline: 1.3306x; 1.2987x over previous
"""Trainium2 Bass kernel for InformationPropagationLayer (GNN message passing).

Computes, per channel c:
    out[c] = 0.5 * h_in[c] + 0.5 * dinv[c] * (A[c] @ x[c] + x[c])
where dinv[c][n] = 1 / (1 + rowsum(A[c])[n])   (== D_tilde_inv @ (A + I) row-scaling)

Strategy:
  - C=16 channels sharded 2-per-core across 8 NeuronCores (no communication).
  - Host-side: A is transposed (so the contraction dim m lands on SBUF
    partitions with fully-contiguous DMA loads) and cast to bf16 (halves the
    dominant HBM traffic; final output error ~1e-5 because the output is
    dominated by the f32 h_in term).
  - x is augmented with a ones column, so a single matmul pass over A yields
    both (A @ x)^T and rowsum(A) (the ones column picks up sum_m A[n, m]).
  - Device: for each channel, accumulate psum[65, 512] = x_aug^T-chunks @ A^T
    over 16 K-chunks (4 psum banks = the 4 n-blocks of 512), then PE-transpose
    back to [n, 65] tiles and apply the fused epilogue on DVE.
"""

import numpy as np
import ml_dtypes
from contextlib import ExitStack

C, N, D = 16, 2048, 64
NCORES = 8
CPC = C // NCORES  # channels per core
P = 128            # SBUF partitions / matmul K
KT = N // P        # 16 contraction chunks
NBLK = 512         # psum bank free-dim (f32)
NBLKS = N // NBLK  # 4
DA = D + 1         # x columns + ones column
BF16 = ml_dtypes.bfloat16
FP8 = ml_dtypes.float8_e4m3fn
A_FP8 = True       # stream A in fp8e4m3 (halves the dominant HBM traffic)

_nc_cache = None


def _build():
    import concourse.mybir as mybir
    import concourse.tile as tile
    from concourse import bacc
    from concourse.masks import make_identity

    nc = bacc.Bacc(
        "TRN2",
        target_bir_lowering=False,
        debug=False,
        enable_asserts=True,
        num_devices=NCORES,
    )
    a_dt = mybir.dt.float8e4 if A_FP8 else mybir.dt.bfloat16
    at = nc.dram_tensor("at", [CPC, N, N], a_dt, kind="ExternalInput").ap()
    xaug = nc.dram_tensor(
        "xaug", [CPC, N, DA], mybir.dt.bfloat16, kind="ExternalInput"
    ).ap()
    hh = nc.dram_tensor("hh", [CPC, N, D], mybir.dt.float32, kind="ExternalInput").ap()
    out = nc.dram_tensor("out", [CPC, N, D], mybir.dt.float32, kind="ExternalOutput").ap()

    f32 = mybir.dt.float32
    bf16 = mybir.dt.bfloat16
    mult = mybir.AluOpType.mult
    add = mybir.AluOpType.add

    with tile.TileContext(nc) as tc, ExitStack() as ctx:
        singles = ctx.enter_context(tc.tile_pool(name="singles", bufs=1))
        xpool = ctx.enter_context(tc.tile_pool(name="xpool", bufs=2))
        hpool = ctx.enter_context(tc.tile_pool(name="hpool", bufs=2))
        opool = ctx.enter_context(tc.tile_pool(name="opool", bufs=2))
        apool = ctx.enter_context(tc.tile_pool(name="apool", bufs=8))
        tpool = ctx.enter_context(tc.tile_pool(name="tpool", bufs=2))
        epool = ctx.enter_context(tc.tile_pool(name="epool", bufs=4))
        spool = ctx.enter_context(tc.tile_pool(name="spool", bufs=8))
        psacc = ctx.enter_context(tc.tile_pool(name="psacc", bufs=4, space="PSUM"))
        pstp = ctx.enter_context(tc.tile_pool(name="pstp", bufs=2, space="PSUM"))

        ident = singles.tile([P, P], f32)
        make_identity(nc, ident)

        # Round-robin big loads across engine HWDGE queues — a single queue
        # caps at ~260 GB/s, well under the core's HBM bandwidth.
        qs = [nc.sync, nc.scalar, nc.gpsimd]

        a_dt_s = mybir.dt.float8e4 if A_FP8 else mybir.dt.bfloat16
        for c in range(CPC):
            xs = xpool.tile([P, KT, DA], bf16, tag="xs")
            nc.scalar.dma_start(out=xs, in_=xaug[c].rearrange("(k p) a -> p k a", p=P))
            hs = hpool.tile([P, KT, D], f32, tag="hs")
            nc.sync.dma_start(out=hs, in_=hh[c].rearrange("(k p) d -> p k d", p=P))
            os_ = opool.tile([P, KT, D], f32, tag="os")

            # 4 psum banks accumulate the 4 n-blocks; A chunks are separate
            # tiles so each matmul only waits on its own chunk's DMA.
            phs = [
                psacc.tile([DA, NBLK], f32, tag="ph", name=f"ph_{c}_{b}")
                for b in range(NBLKS)
            ]
            for ki in range(KT):
                a_sb = apool.tile([P, N], a_dt_s, tag="a")
                qs[ki % len(qs)].dma_start(
                    out=a_sb, in_=at[c, ki * P : (ki + 1) * P, :]
                )
                for b in range(NBLKS):
                    nc.tensor.matmul(
                        phs[b][:, :],
                        xs[:, ki, :],
                        a_sb[:, b * NBLK : (b + 1) * NBLK],
                        start=(ki == 0),
                        stop=(ki == KT - 1),
                    )
            for b in range(NBLKS):
                hT = tpool.tile([DA, NBLK], f32, tag="hT")
                nc.vector.tensor_copy(hT[:, :], phs[b][:, :])
                # 4 transposes land in one psum bank -> wide batched epilogue
                pt4 = pstp.tile([P, NBLK // P, DA], f32, tag="pt4")
                for j in range(NBLK // P):
                    nc.tensor.transpose(
                        pt4[:, j, :], hT[:, j * P : (j + 1) * P], ident[:DA, :DA]
                    )
                t0, t1 = b * (NBLK // P), (b + 1) * (NBLK // P)
                # rcp = 1/(2 + 2*rowsum) = 0.5*dinv  (per (partition, j))
                den = spool.tile([P, NBLK // P], f32, tag="den")
                nc.vector.tensor_scalar(
                    den, pt4[:, :, D], 2.0, 2.0, op0=mult, op1=add
                )
                rcp = spool.tile([P, NBLK // P], f32, tag="rcp")
                nc.vector.reciprocal(rcp, den)
                rcp_b = rcp[:, :].broadcast_to([P, NBLK // P, D])
                # hx = (A@x + x) * (0.5*dinv) ; out = hx + 0.5*h_in
                hx = epool.tile([P, NBLK // P, D], f32, tag="hx")
                nc.vector.tensor_add(hx, pt4[:, :, 0:D], xs[:, t0:t1, 0:D])
                nc.vector.tensor_mul(hx, hx, rcp_b)
                nc.vector.tensor_add(os_[:, t0:t1, :], hx, hs[:, t0:t1, :])
            nc.gpsimd.dma_start(out=out[c].rearrange("(k p) d -> p k d", p=P), in_=os_)

    nc.compile()
    return nc


def _get_nc():
    global _nc_cache
    if _nc_cache is None:
        _nc_cache = _build()
    return _nc_cache


def _make_in_maps(x, h_in, A_inter):
    x = np.asarray(x, dtype=np.float32)
    h_in = np.asarray(h_in, dtype=np.float32)
    A = np.asarray(A_inter, dtype=np.float32)
    ones = np.ones((CPC, N, 1), dtype=np.float32)
    in_maps = []
    for k in range(NCORES):
        cs = slice(CPC * k, CPC * (k + 1))
        at = A[cs].transpose(0, 2, 1).astype(FP8 if A_FP8 else BF16)
        xa = np.concatenate([x[cs], ones], axis=2).astype(BF16)
        hhalf = (0.5 * h_in[cs]).astype(np.float32)
        in_maps.append({"at": at, "xaug": xa, "hh": hhalf})
    return in_maps


def _run(in_maps, **kwargs):
    from concourse.bass_utils import run_bass_kernel_spmd

    nc = _get_nc()
    return run_bass_kernel_spmd(nc, in_maps, core_ids=list(range(NCORES)), **kwargs)


def kernel(x, h_in, A_inter):
    res = _run(_make_in_maps(x, h_in, A_inter))
    out = np.concatenate([res.results[k]["out"] for k in range(NCORES)], axis=0)
    return np.ascontiguousarray(out.astype(np.float32))


# revision 15
# speedup vs baseline: 1.4982x; 1.1260x over previous
"""Trainium2 Bass kernel for InformationPropagationLayer (GNN message passing).

Computes, per channel c:
    out[c] = 0.5 * h_in[c] + 0.5 * dinv[c] * (A[c] @ x[c] + x[c])
where dinv[c][n] = 1 / (1 + rowsum(A[c])[n])   (== D_tilde_inv @ (A + I) row-scaling)

Strategy:
  - C=16 channels sharded 2-per-core across 8 NeuronCores (no communication).
  - Host-side: A is transposed (so the contraction dim m lands on SBUF
    partitions with fully-contiguous DMA loads) and cast to bf16 (halves the
    dominant HBM traffic; final output error ~1e-5 because the output is
    dominated by the f32 h_in term).
  - x is augmented with a ones column, so a single matmul pass over A yields
    both (A @ x)^T and rowsum(A) (the ones column picks up sum_m A[n, m]).
  - Device: for each channel, accumulate psum[65, 512] = x_aug^T-chunks @ A^T
    over 16 K-chunks (4 psum banks = the 4 n-blocks of 512), then PE-transpose
    back to [n, 65] tiles and apply the fused epilogue on DVE.
"""

import numpy as np
import ml_dtypes
from contextlib import ExitStack

C, N, D = 16, 2048, 64
NCORES = 8
CPC = C // NCORES  # channels per core
P = 128            # SBUF partitions / matmul K
KT = N // P        # 16 contraction chunks
NBLK = 512         # psum bank free-dim (f32)
NBLKS = N // NBLK  # 4
DA = D + 1         # x columns + ones column
BF16 = ml_dtypes.bfloat16
FP8 = ml_dtypes.float8_e4m3fn
A_FP8 = True       # stream A in fp8e4m3 (halves the dominant HBM traffic)

_nc_cache = None


def _build():
    import concourse.mybir as mybir
    import concourse.tile as tile
    from concourse import bacc
    from concourse.masks import make_identity

    nc = bacc.Bacc(
        "TRN2",
        target_bir_lowering=False,
        debug=False,
        enable_asserts=True,
        num_devices=NCORES,
    )
    a_dt = mybir.dt.float8e4 if A_FP8 else mybir.dt.bfloat16
    at = nc.dram_tensor("at", [CPC, N, N], a_dt, kind="ExternalInput").ap()
    xaug = nc.dram_tensor(
        "xaug", [CPC, N, DA], mybir.dt.bfloat16, kind="ExternalInput"
    ).ap()
    hh = nc.dram_tensor("hh", [CPC, N, D], mybir.dt.float32, kind="ExternalInput").ap()
    out = nc.dram_tensor("out", [CPC, N, D], mybir.dt.float32, kind="ExternalOutput").ap()

    f32 = mybir.dt.float32
    bf16 = mybir.dt.bfloat16
    mult = mybir.AluOpType.mult
    add = mybir.AluOpType.add

    with tile.TileContext(nc) as tc, ExitStack() as ctx:
        singles = ctx.enter_context(tc.tile_pool(name="singles", bufs=1))
        xpool = ctx.enter_context(tc.tile_pool(name="xpool", bufs=2))
        hpool = ctx.enter_context(tc.tile_pool(name="hpool", bufs=2))
        opool = ctx.enter_context(tc.tile_pool(name="opool", bufs=2))
        apool = ctx.enter_context(tc.tile_pool(name="apool", bufs=12))
        tpool = ctx.enter_context(tc.tile_pool(name="tpool", bufs=2))
        epool = ctx.enter_context(tc.tile_pool(name="epool", bufs=4))
        spool = ctx.enter_context(tc.tile_pool(name="spool", bufs=8))
        psacc = ctx.enter_context(tc.tile_pool(name="psacc", bufs=4, space="PSUM"))
        pstp = ctx.enter_context(tc.tile_pool(name="pstp", bufs=2, space="PSUM"))

        ident = singles.tile([P, P], f32)
        make_identity(nc, ident)

        # Round-robin big loads across engine HWDGE queues — a single queue
        # caps at ~260 GB/s, well under the core's HBM bandwidth.
        qs = [nc.sync, nc.scalar, nc.gpsimd]

        a_dt_s = mybir.dt.float8e4 if A_FP8 else mybir.dt.bfloat16

        # Small per-channel tensors for BOTH channels load up-front so the
        # first matmuls and epilogues never queue behind bulk A traffic.
        xss, hss, oss = [], [], []
        for c in range(CPC):
            xs = xpool.tile([P, KT, DA], bf16, tag="xs", name=f"xs_{c}")
            nc.sync.dma_start(out=xs, in_=xaug[c].rearrange("(k p) a -> p k a", p=P))
            hs = hpool.tile([P, KT, D], f32, tag="hs", name=f"hs_{c}")
            nc.scalar.dma_start(out=hs, in_=hh[c].rearrange("(k p) d -> p k d", p=P))
            xss.append(xs)
            hss.append(hs)
            oss.append(opool.tile([P, KT, D], f32, tag="os", name=f"os_{c}"))

        for c in range(CPC):
            xs, hs, os_ = xss[c], hss[c], oss[c]
            # 4 psum banks accumulate the 4 n-blocks; A chunks are separate
            # tiles so each matmul only waits on its own chunk's DMA.
            phs = [
                psacc.tile([DA, NBLK], f32, tag="ph", name=f"ph_{c}_{b}")
                for b in range(NBLKS)
            ]
            for ki in range(KT):
                a_sb = apool.tile([P, N], a_dt_s, tag="a")
                qs[ki % len(qs)].dma_start(
                    out=a_sb, in_=at[c, ki * P : (ki + 1) * P, :]
                )
                for b in range(NBLKS):
                    nc.tensor.matmul(
                        phs[b][:, :],
                        xs[:, ki, :],
                        a_sb[:, b * NBLK : (b + 1) * NBLK],
                        start=(ki == 0),
                        stop=(ki == KT - 1),
                    )
            for b in range(NBLKS):
                hT = tpool.tile([DA, NBLK], f32, tag="hT")
                nc.vector.tensor_copy(hT[:, :], phs[b][:, :])
                # 4 transposes land in one psum bank -> wide batched epilogue
                pt4 = pstp.tile([P, NBLK // P, DA], f32, tag="pt4")
                for j in range(NBLK // P):
                    nc.tensor.transpose(
                        pt4[:, j, :], hT[:, j * P : (j + 1) * P], ident[:DA, :DA]
                    )
                t0, t1 = b * (NBLK // P), (b + 1) * (NBLK // P)
                # rcp = 1/(2 + 2*rowsum) = 0.5*dinv  (per (partition, j))
                den = spool.tile([P, NBLK // P], f32, tag="den")
                nc.vector.tensor_scalar(
                    den, pt4[:, :, D], 2.0, 2.0, op0=mult, op1=add
                )
                rcp = spool.tile([P, NBLK // P], f32, tag="rcp")
                nc.vector.reciprocal(rcp, den)
                rcp_b = rcp[:, :].broadcast_to([P, NBLK // P, D])
                # hx = (A@x + x) * (0.5*dinv) ; out = hx + 0.5*h_in
                hx = epool.tile([P, NBLK // P, D], f32, tag="hx")
                nc.vector.tensor_add(hx, pt4[:, :, 0:D], xs[:, t0:t1, 0:D])
                nc.vector.tensor_mul(hx, hx, rcp_b)
                nc.vector.tensor_add(os_[:, t0:t1, :], hx, hs[:, t0:t1, :])
                # stream each n-block's result out as soon as it's ready
                nc.gpsimd.dma_start(
                    out=out[c].rearrange("(k p) d -> p k d", p=P)[:, t0:t1, :],
                    in_=os_[:, t0:t1, :],
                )

    nc.compile()
    return nc


def _get_nc():
    global _nc_cache
    if _nc_cache is None:
        _nc_cache = _build()
    return _nc_cache


def _make_in_maps(x, h_in, A_inter):
    x = np.asarray(x, dtype=np.float32)
    h_in = np.asarray(h_in, dtype=np.float32)
    A = np.asarray(A_inter, dtype=np.float32)
    ones = np.ones((CPC, N, 1), dtype=np.float32)
    in_maps = []
    for k in range(NCORES):
        cs = slice(CPC * k, CPC * (k + 1))
        at = A[cs].transpose(0, 2, 1).astype(FP8 if A_FP8 else BF16)
        xa = np.concatenate([x[cs], ones], axis=2).astype(BF16)
        hhalf = (0.5 * h_in[cs]).astype(np.float32)
        in_maps.append({"at": at, "xaug": xa, "hh": hhalf})
    return in_maps


def _run(in_maps, **kwargs):
    from concourse.bass_utils import run_bass_kernel_spmd

    nc = _get_nc()
    return run_bass_kernel_spmd(nc, in_maps, core_ids=list(range(NCORES)), **kwargs)


def kernel(x, h_in, A_inter):
    res = _run(_make_in_maps(x, h_in, A_inter))
    out = np.concatenate([res.results[k]["out"] for k in range(NCORES)], axis=0)
    return np.ascontiguousarray(out.astype(np.float32))
